# revision 15
# baseline (speedup 1.0000x reference)
"""Trainium2 Bass kernel for nn_PreprocessingLayer (landmark preprocessing).

Input  x:  (256, 384, 543, 3) f32
Output:    (256, 384, 744) f32

Sharding: pure data parallel, batch 256 -> 32 per core across 8 cores.

Per-core layout: frames on partitions (3 blocks of 128), features on the
free dim, NB=2 batches processed side-by-side per iteration, gathers staged
per group of NG batches to amortize DMA instruction overhead. Only the 62
used landmarks are read from HBM. Frame-shifted copies of the normalized
coordinates are produced with SBUF->SBUF partition-shifted DMAs; tail rows
are duplicated so the padded frames come out as exact zeros.
"""

import math
from contextlib import ExitStack

import numpy as np

import concourse.bacc as bacc
import concourse.tile as tile
from concourse import mybir
from concourse.bass_utils import run_bass_kernel_spmd

F32 = mybir.dt.float32
A = mybir.AluOpType
AF = mybir.ActivationFunctionType

_FACE = [33, 133, 362, 263, 61, 291, 199, 419, 17, 84,
         17, 314, 405, 320, 307, 375, 321, 308, 324, 318]
S = 384          # frames
K = 3            # frame blocks of 128
P = 128
NL = 62          # landmarks kept
NF = 744         # output features per frame
EPS = 1e-8
N_STD = S * NL   # population size for std (includes duplicated lm 17)
PI = math.pi


def kernel_body(tc, out_h, x_h, B_loc, NB=2, NG=8):
    nc = tc.nc
    x_ap = x_h[:]
    out_ap = out_h[:]
    NG = min(NG, B_loc)
    n_groups = B_loc // NG
    pairs_per_group = NG // NB
    W = NL * 2          # 124 xy floats per frame
    WR = NL * 3         # 186 raw floats per frame (with z)
    Q = NB * K * NL     # per-pair landmark-feature count (372)

    with ExitStack() as ctx:
        pconst = ctx.enter_context(tc.tile_pool(name="const", bufs=1))
        pool = ctx.enter_context(tc.tile_pool(name="work", bufs=2))
        psum = ctx.enter_context(tc.tile_pool(name="ps", bufs=2, space="PSUM"))

        ones_col = pconst.tile([P, 1], F32)
        nc.vector.memset(ones_col[:], 1.0)
        ones_row = pconst.tile([1, P], F32)
        nc.vector.memset(ones_row[:], 1.0)
        eps_bias = pconst.tile([P, 1], F32)
        nc.vector.memset(eps_bias[:], 1e-32)

        def tmp(tag):
            return pool.tile([P, Q], F32, name=tag, tag=tag)

        for g in range(n_groups):
            g0 = g * NG
            xs = x_ap[g0:g0 + NG]                    # [NG, S, 543, 3]

            # ---------------- gather (whole group) ----------------
            st = pool.tile([P, NG * K * WR], F32, name="st")
            vg = st[:].rearrange("p (b k f) -> p b k f", b=NG, k=K)
            # hands: landmarks 501..542 -> slots 20..61 (contiguous span)
            src = xs[:, :, 501:543, :].rearrange(
                "b (k p) l c -> p b k (l c)", k=K, p=P)
            nc.scalar.dma_start(vg[:, :, :, 60:186], src)
            # face: 20 single-landmark spans -> slots 0..19
            for j, lm in enumerate(_FACE):
                srcf = xs[:, :, lm:lm + 1, :].rearrange(
                    "b (k p) l c -> p b k (l c)", k=K, p=P)
                eng = nc.sync if j % 2 == 0 else nc.scalar
                eng.dma_start(vg[:, :, :, 3 * j:3 * j + 3], srcf)

            for it in range(pairs_per_group):
                bg = it * NB                         # batch offset in group
                b0 = g0 + bg
                v0 = vg[:, bg:bg + NB]               # [p, NB, K, WR]

                # ---------------- stats ----------------
                sqraw = pool.tile([P, NB * K * WR], F32, name="sqraw")
                nc.scalar.activation(
                    sqraw[:].rearrange("p (b k f) -> p b k f", b=NB, k=K),
                    v0, AF.Square)
                vq = sqraw[:].rearrange("p (b k f) -> p b k f", b=NB, k=K)

                psum_s = psum.tile([1, NB * WR], F32, name="psum_s")
                psum_q = psum.tile([1, NB * WR], F32, name="psum_q")
                for b in range(NB):
                    for k in range(K):
                        nc.tensor.matmul(psum_s[:, b * WR:(b + 1) * WR],
                                         ones_col[:], v0[:, b, k, :],
                                         start=(k == 0), stop=(k == K - 1))
                        nc.tensor.matmul(psum_q[:, b * WR:(b + 1) * WR],
                                         ones_col[:], vq[:, b, k, :],
                                         start=(k == 0), stop=(k == K - 1))

                # landmark sums over x,y per (b,c), straight from PSUM
                lmsum = pool.tile([1, NB * 2], F32, name="lmsum")
                vsum = psum_s[:].rearrange("p (b l c) -> p b c l",
                                           b=NB, c=3)
                nc.vector.tensor_reduce(
                    out=lmsum[:].rearrange("p (b c) -> p b c", b=NB),
                    in_=vsum[:, :, 0:2, :], axis=mybir.AxisListType.X,
                    op=A.add)
                lmsq = pool.tile([1, NB * 2], F32, name="lmsq")
                vsq2 = psum_q[:].rearrange("p (b l c) -> p b c l",
                                           b=NB, c=3)
                nc.vector.tensor_reduce(
                    out=lmsq[:].rearrange("p (b c) -> p b c", b=NB),
                    in_=vsq2[:, :, 0:2, :], axis=mybir.AxisListType.X,
                    op=A.add)

                # nose mean (slot 8): m = nose_sum / S
                stats_row = pool.tile([1, NB * 4], F32, name="stats_row")
                nose_v = psum_s[:].rearrange(
                    "p (b f) -> p b f", b=NB)[:, :, 24:26]
                mvals = pool.tile([1, NB * 2], F32, name="mvals")
                nc.vector.tensor_scalar(
                    out=mvals[:].rearrange("p (b c) -> p b c", b=NB),
                    in0=nose_v, scalar1=1.0 / S, scalar2=None, op0=A.mult)

                # var = (sumsq - sum^2/N)/(N-1); inv = 1/(sqrt(var)+eps)
                s2 = pool.tile([1, NB * 2], F32, name="s2")
                nc.vector.tensor_tensor(out=s2[:], in0=lmsum[:],
                                        in1=lmsum[:], op=A.mult)
                v1 = pool.tile([1, NB * 2], F32, name="v1")
                nc.vector.scalar_tensor_tensor(
                    out=v1[:], in0=s2[:], scalar=-1.0 / N_STD, in1=lmsq[:],
                    op0=A.mult, op1=A.add)
                stdv = pool.tile([1, NB * 2], F32, name="stdv")
                nc.scalar.activation(stdv[:], v1[:], AF.Sqrt,
                                     scale=1.0 / (N_STD - 1))
                stdp = pool.tile([1, NB * 2], F32, name="stdp")
                nc.vector.tensor_scalar(out=stdp[:], in0=stdv[:],
                                        scalar1=EPS, scalar2=None,
                                        op0=A.add)
                nc.vector.reciprocal(stats_row[:, NB * 2:NB * 4], stdp[:])
                nc.vector.tensor_copy(stats_row[:, 0:NB * 2], mvals[:])

                bc_ps = psum.tile([P, NB * 4], F32, name="bc_ps")
                nc.tensor.matmul(bc_ps[:], ones_row[:], stats_row[:],
                                 start=True, stop=True)
                bc = pool.tile([P, NB * 4], F32, name="bc")
                nc.vector.tensor_copy(bc[:], bc_ps[:])

                # ---------------- normalize into OT ----------------
                # xy0 = (x - m) * inv with m/inv broadcast from bc
                ot = pool.tile([P, NB * K * NF], F32, name="ot")
                vot = ot[:].rearrange("p (b k f) -> p b k f", b=NB, k=K)
                vbc = bc[:].rearrange("p (h b c) -> p h b c", h=2, b=NB)
                mv = vbc[:, 0].rearrange("p b (o c) -> p b o c", o=1)
                iv = vbc[:, 1].rearrange("p b (o c) -> p b o c", o=1)
                mbc = mv.broadcast_to([P, NB, K * NL, 2])
                ibc = iv.broadcast_to([P, NB, K * NL, 2])
                st_xy = v0.rearrange("p b k (l c) -> p b (k l) c",
                                     c=3)[:, :, :, 0:2]
                tmp744 = pool.tile([P, NB * K * W], F32, name="tmp744")
                vt744 = tmp744[:].rearrange("p (b q c) -> p b q c",
                                            b=NB, c=2)
                nc.vector.tensor_tensor(out=vt744, in0=st_xy, in1=mbc,
                                        op=A.subtract)
                for b in range(NB):
                    oxy_b = vot[:, b, :, 0:W].rearrange(
                        "p k (l c) -> p k l c", c=2)
                    t_b = tmp744[:].rearrange(
                        "p (b k l c) -> p b k l c", b=NB, k=K, c=2)[:, b]
                    i_b = vbc[:, 1, b].rearrange(
                        "p (o q c) -> p o q c", o=1,
                        q=1).broadcast_to([P, K, NL, 2])
                    nc.vector.tensor_tensor(out=oxy_b, in0=t_b, in1=i_b,
                                            op=A.mult)

                vxy = vot[:, :, :, 0:W]

                # ------------- frame-shifted copies (SWDGE) -------------
                s1i = pool.tile([P, NB * K * W], F32, name="s1i")
                vs1 = s1i[:].rearrange("p (b k f) -> p b k f", b=NB, k=K)
                nc.sync.dma_start(vs1[0:P - 1], vxy[1:P])
                for k in range(K - 1):
                    nc.gpsimd.dma_start(vs1[P - 1:P, :, k, :],
                                        vxy[0:1, :, k + 1, :])
                nc.gpsimd.dma_start(vs1[P - 1:P, :, K - 1, :],
                                    vxy[P - 1:P, :, K - 1, :])

                s2i = pool.tile([P, NB * K * W], F32, name="s2i")
                vs2 = s2i[:].rearrange("p (b k f) -> p b k f", b=NB, k=K)
                nc.sync.dma_start(vs2[0:P - 2], vxy[2:P])
                for k in range(K - 1):
                    nc.gpsimd.dma_start(vs2[P - 2:P, :, k, :],
                                        vxy[0:2, :, k + 1, :])
                nc.gpsimd.dma_start(vs2[P - 2:P, :, K - 1, :],
                                    vxy[P - 2:P, :, K - 1, :])

                # ---------------- diffs ----------------
                vdx = vot[:, :, :, W:2 * W]
                nc.vector.tensor_tensor(out=vdx, in0=vs1[:], in1=vxy,
                                        op=A.subtract)
                vdx2 = vot[:, :, :, 2 * W:3 * W]
                nc.vector.tensor_tensor(out=vdx2, in0=vs2[:], in1=vxy,
                                        op=A.subtract)
                dni = pool.tile([P, NB * K * W], F32, name="dni")
                vdn = dni[:].rearrange("p (b k f) -> p b k f", b=NB, k=K)
                nc.vector.tensor_tensor(out=vdn, in0=vs2[:], in1=vs1[:],
                                        op=A.subtract)

                # ---------------- relative motion ----------------
                nc.gpsimd.memset(
                    vot[:, :, :, 3 * W + 2 * NL - 2:3 * W + 2 * NL], 0.0)
                nc.gpsimd.memset(vot[:, :, K - 1, 3 * W:4 * W], 0.0)
                nc.vector.tensor_tensor(
                    out=vot[:, :, 0:K - 1, 3 * W:4 * W - 2],
                    in0=vs1[:, :, 0:K - 1, 0:W - 2],
                    in1=vxy[:, :, 0:K - 1, 2:W], op=A.subtract)
                nc.vector.tensor_tensor(
                    out=vot[0:P - 1, :, K - 1, 3 * W:4 * W - 2],
                    in0=vs1[0:P - 1, :, K - 1, 0:W - 2],
                    in1=vxy[0:P - 1, :, K - 1, 2:W], op=A.subtract)

                # ---------------- cosine similarity ----------------
                prodt = pool.tile([P, NB * K * W], F32, name="prodt")
                nc.vector.tensor_tensor(
                    out=prodt[:].rearrange("p (b k f) -> p b k f",
                                           b=NB, k=K),
                    in0=vdx, in1=vdn, op=A.mult)
                sq1t = pool.tile([P, NB * K * W], F32, name="sq1t")
                nc.scalar.activation(
                    sq1t[:].rearrange("p (b k f) -> p b k f", b=NB, k=K),
                    vdx, AF.Square)
                sq2t = pool.tile([P, NB * K * W], F32, name="sq2t")
                nc.scalar.activation(sq2t[:], dni[:], AF.Square)

                def pair_sum(dst, srct):
                    v = srct[:].rearrange("p (q c) -> p c q", c=2)
                    nc.vector.tensor_tensor(out=dst[:], in0=v[:, 0],
                                            in1=v[:, 1], op=A.add)

                adott = tmp("adott")
                pair_sum(adott, prodt)
                na2t = pool.tile([P, Q], F32, name="na2t")
                pair_sum(na2t, sq1t)
                nb2t = tmp("nb2t")
                pair_sum(nb2t, sq2t)

                den2t = tmp("den2t")
                nc.vector.tensor_tensor(out=den2t[:], in0=na2t[:],
                                        in1=nb2t[:], op=A.mult)
                dent = tmp("dent")
                nc.scalar.activation(dent[:], den2t[:], AF.Sqrt,
                                     bias=eps_bias[:])
                rct = tmp("rct")
                nc.vector.reciprocal(rct[:], dent[:])
                cosvt = pool.tile([P, Q], F32, name="cosvt")
                nc.vector.tensor_tensor(out=cosvt[:], in0=adott[:],
                                        in1=rct[:], op=A.mult)

                # temporal consistency: duplicate cos into (l, c) pairs
                vcos = cosvt[:].rearrange("p (b k l) -> p b k l",
                                          b=NB, k=K)
                nc.gpsimd.memset(vot[:, :, K - 1, 4 * W:5 * W], 0.0)
                for c in range(2):
                    otc = vot[:, :, :, 4 * W:5 * W].rearrange(
                        "p b k (l c) -> p c b k l", c=2)[:, c]
                    nc.gpsimd.tensor_copy(otc[:, :, 0:K - 1],
                                          vcos[:, :, 0:K - 1])
                    nc.gpsimd.tensor_copy(otc[0:P - 2, :, K - 1],
                                          vcos[0:P - 2, :, K - 1])

                # ---------------- motion magnitude ----------------
                vmm = vot[:, :, :, 5 * W:5 * W + NL]
                nc.scalar.activation(
                    vmm,
                    na2t[:].rearrange("p (b k l) -> p b k l", b=NB, k=K),
                    AF.Sqrt)

                # ---------------- motion direction (atan2) ----------------
                abst = pool.tile([P, NB * K * W], F32, name="abst")
                nc.scalar.activation(
                    abst[:].rearrange("p (b k f) -> p b k f", b=NB, k=K),
                    vdx, AF.Abs)
                vab = abst[:].rearrange("p (q c) -> p c q", c=2)
                ax_v, ay_v = vab[:, 0], vab[:, 1]
                numt = tmp("numt")
                nc.vector.tensor_tensor(out=numt[:], in0=ay_v, in1=ax_v,
                                        op=A.subtract)
                dent2 = tmp("dent2")
                nc.vector.scalar_tensor_tensor(out=dent2[:], in0=ay_v,
                                               scalar=1e-30, in1=ax_v,
                                               op0=A.add, op1=A.add)
                rcd = tmp("rcd")
                nc.vector.reciprocal(rcd[:], dent2[:])
                qt = tmp("qt")
                nc.vector.tensor_tensor(out=qt[:], in0=numt[:],
                                        in1=rcd[:], op=A.mult)
                dt_t = tmp("dt_t")
                nc.scalar.activation(dt_t[:], qt[:], AF.Arctan)

                vdx_x = vdx.rearrange("p b k (l c) -> p c b k l", c=2)[:, 0]
                vdx_y = vdx.rearrange("p b k (l c) -> p c b k l", c=2)[:, 1]
                ngt = tmp("ngt")
                nc.vector.tensor_scalar(
                    out=ngt[:].rearrange("p (b k l) -> p b k l",
                                         b=NB, k=K),
                    in0=vdx_x, scalar1=0.0, scalar2=None, op0=A.is_lt)
                w0t = tmp("w0t")
                nc.vector.tensor_tensor(out=w0t[:], in0=dt_t[:],
                                        in1=ngt[:], op=A.mult)
                bt = tmp("bt")
                nc.vector.scalar_tensor_tensor(out=bt[:], in0=w0t[:],
                                               scalar=-2.0, in1=dt_t[:],
                                               op0=A.mult, op1=A.add)
                ct = tmp("ct")
                nc.vector.scalar_tensor_tensor(out=ct[:], in0=ngt[:],
                                               scalar=PI / 2, in1=bt[:],
                                               op0=A.mult, op1=A.add)
                # sign with +0 -> +1 (matches atan2(+0, x<0) = pi)
                sgt = tmp("sgt")
                nc.vector.tensor_scalar(
                    out=sgt[:].rearrange("p (b k l) -> p b k l",
                                         b=NB, k=K),
                    in0=vdx_y, scalar1=0.0, scalar2=2.0,
                    op0=A.is_ge, op1=A.mult)
                sg2 = tmp("sg2")
                nc.vector.tensor_scalar(out=sg2[:], in0=sgt[:],
                                        scalar1=1.0, scalar2=None,
                                        op0=A.subtract)
                dvt = tmp("dvt")
                nc.vector.tensor_tensor(out=dvt[:], in0=ct[:], in1=sg2[:],
                                        op=A.mult)
                vdir = vot[:, :, :, 5 * W + NL:6 * W]
                nc.gpsimd.memset(vot[:, :, K - 1, 5 * W + NL:6 * W], 0.0)
                sg4 = sg2[:].rearrange("p (b k l) -> p b k l", b=NB, k=K)
                dv4 = dvt[:].rearrange("p (b k l) -> p b k l", b=NB, k=K)
                for b in range(NB):
                    nc.vector.scalar_tensor_tensor(
                        out=vdir[:, b, 0:K - 1], in0=sg4[:, b, 0:K - 1],
                        scalar=PI / 4, in1=dv4[:, b, 0:K - 1],
                        op0=A.mult, op1=A.add)
                nc.vector.scalar_tensor_tensor(
                    out=vdir[0:P - 1, :, K - 1], in0=sg4[0:P - 1, :, K - 1],
                    scalar=PI / 4, in1=dv4[0:P - 1, :, K - 1],
                    op0=A.mult, op1=A.add)

                # ---------------- store ----------------
                dst = out_ap[b0:b0 + NB].rearrange("b (k p) f -> p b k f",
                                                   k=K, p=P)
                nc.sync.dma_start(dst, vot)


def build_kernel(B_loc=32, NB=2, NG=8):
    nc = bacc.Bacc()
    x_h = nc.declare_dram_parameter("x", [B_loc, S, 543, 3], F32,
                                    isOutput=False)
    out_h = nc.declare_dram_parameter("out", [B_loc, S, NF], F32,
                                      isOutput=True)
    with tile.TileContext(nc) as tc:
        kernel_body(tc, out_h, x_h, B_loc, NB, NG)
    nc.finalize()
    return nc


_CACHED = {}


def kernel(x, trace=False):
    B = x.shape[0]
    n_cores = 8
    b_loc = B // n_cores
    if b_loc not in _CACHED:
        _CACHED[b_loc] = build_kernel(B_loc=b_loc)
    nc = _CACHED[b_loc]
    x = np.ascontiguousarray(np.asarray(x), dtype=np.float32)
    in_maps = [{"x": x[b_loc * i:b_loc * (i + 1)]} for i in range(n_cores)]
    res = run_bass_kernel_spmd(nc, in_maps, list(range(n_cores)),
                               trace=trace)
    out = np.concatenate([res.results[i]["out"] for i in range(n_cores)],
                         axis=0)
    if trace:
        kernel.last_result = res
    return out
